# revision 19
# baseline (speedup 1.0000x reference)
"""Trainium2 Bass kernel for the DataReloadingQNN problem.

Math: layers 0..4 plus the shared RZ/RY/RZ of layer 5 collapse into one
fixed state w (params only).  The data gates are RY(x_q) = c_q I + s_q J_q
with J a signed permutation, all commuting.  Peel qubits 8,9,10: contract
the other eight via a dense matmul T = W_lo @ U with
    W_lo[b, m] = tensor product of 8 [cos,sin] pairs  (m in [0,256)),
    U[m, :]   = (P J^{(m)} w) re/im-interleaved, P = CNOT chain,
then apply per peeled qubit the per-sample rotation
    T <- c_q T + s_q * sign_q ( T[col ^ M_q] ).
Columns are relabeled host-side by a linear GF(2) map chosen so each
peeled rotation is a single y-bit flip with sign = that bit (bits 4,3,2
-> col blocks 64/32/16, contiguous halves).  The q=8 rotation is folded
into the matmul (signed-permuted copy U8, weight variants c8*W and s8*W
-> K=512); q=9 splits as two ScalarE scaled PSUM->SBUF copies (c9*, s9*)
plus two VectorE tensor_tensor half-adds; q=10 as two VectorE
tensor_scalar (4x) plus two tensor_tensor (2x).  W transposes go through
the DMA crossbar, keeping TensorE purely on matmuls.  Output is written
bf16 in y-order; the host upcasts and unpermutes.

Per core: 1024 samples = 8 tiles of 128.  Inputs sharded batch-wise
across 8 cores; U replicated.
"""
import numpy as np
import ml_dtypes

import concourse.bass as bass
import concourse.bacc as bacc
import concourse.tile as tile
from concourse import mybir
from concourse.bass_utils import run_bass_kernel_spmd

N = 11
DIM = 2048
BATCH = 8192
NCORES = 8
BSH = BATCH // NCORES          # 1024 samples per core
NTILES = BSH // 128            # 8 sample-tiles per core
KLO = 8                        # qubits contracted in the matmul
NU = 1 << KLO                  # 256 rows of U
W2 = 2 * DIM                   # 4096 output columns (re/im interleaved)
DC = 1024                      # double-chunk width (2 PSUM banks)
NDC = W2 // DC                 # 4 double-chunks
F32 = mybir.dt.float32
BF16 = mybir.dt.bfloat16

MUL = mybir.AluOpType.mult
ADD = mybir.AluOpType.add
SUB = mybir.AluOpType.subtract

# ---------------------------------------------------------------- host math


def _rz(phi):
    e = np.exp(-0.5j * phi)
    return np.array([[e, 0], [0, np.conj(e)]], dtype=np.complex128)


def _ry(theta):
    t = 0.5 * theta
    c, s = np.cos(t), np.sin(t)
    return np.array([[c, -s], [s, c]], dtype=np.complex128)


def _apply_1q_rows(rows, U, q):
    R = rows.shape[0]
    st = rows.reshape(R, 2 ** q, 2, 2 ** (N - 1 - q))
    st = np.einsum('ab,rxby->rxay', U, st)
    return st.reshape(R, DIM)


def _apply_cnot_rows(rows, c):
    R = rows.shape[0]
    st = rows.reshape(R, 2 ** c, 2, 2, 2 ** (N - 2 - c))
    st = np.stack([st[:, :, 0], st[:, :, 1, ::-1]], axis=2)
    return st.reshape(R, DIM)


def _y_of_x():
    """Column relabeling y = R x: y0=x3, y1=x4, y2=x0^x1, y3=x1^x2,
    y4=x2^x3, y5..10 = x5..x10 (bit i of the state index = 2^i)."""
    x = np.arange(DIM)
    x0, x1 = x & 1, (x >> 1) & 1
    x2, x3 = (x >> 2) & 1, (x >> 3) & 1
    x4 = (x >> 4) & 1
    return ((x & ~np.int64(31)) | (x3 << 0) | (x4 << 1)
            | ((x0 ^ x1) << 2) | ((x1 ^ x2) << 3) | ((x2 ^ x3) << 4))


def _x_of_y():
    y = _y_of_x()
    inv = np.empty(DIM, dtype=np.int64)
    inv[y] = np.arange(DIM)
    return inv


def build_u_matrices(params):
    """(6,11,3) f32 -> (Uy, U8), each (256, 4096) f64 in y-order.
    U8 is the signed bit-4-flip permutation of Uy (folds the q=8 gate)."""
    p = params.astype(np.float64)
    v = np.zeros((1, DIM), dtype=np.complex128)
    v[0, 0] = 1.0
    for l in range(5):
        for q in range(N):
            v = _apply_1q_rows(v, _rz(p[l, q, 0]), q)
            v = _apply_1q_rows(v, _ry(p[l, q, 1]), q)
            v = _apply_1q_rows(v, _rz(p[l, q, 2]), q)
        for c in range(N - 1):
            v = _apply_cnot_rows(v, c)
    for q in range(N):
        B = _rz(p[5, q, 2]) @ _ry(p[5, q, 1]) @ _rz(p[5, q, 0])
        v = _apply_1q_rows(v, B, q)

    # rows over J-subsets of qubits 0..7 (bit b of m <-> qubit b)
    rows = v
    idx = np.arange(DIM)
    for q in range(KLO):
        m = 1 << (N - 1 - q)
        sgn = np.where(idx & m, 1.0, -1.0)
        rows = np.concatenate([rows, sgn * rows[:, idx ^ m]], axis=0)

    # fold CNOT-chain permutation, then relabel columns to y-order
    g = np.arange(DIM)[None, :]
    for c in range(N - 1):
        g = _apply_cnot_rows(g.astype(np.float64), c).astype(np.int64)
    rows = rows[:, g[0]][:, _x_of_y()]

    # fold the q=8 rotation: U8 = sign(y bit 4) * Uy[:, y ^ 16]
    yy = np.arange(DIM)
    sgn8 = np.where((yy >> 4) & 1, 1.0, -1.0)
    rows8 = sgn8[None, :] * rows[:, yy ^ 16]

    def interleave(r):
        U = np.empty((NU, W2), dtype=np.float64)
        U[:, 0::2] = r.real
        U[:, 1::2] = r.imag
        return U

    return interleave(rows), interleave(rows8)


# ------------------------------------------------------------- bass kernel


def _rot_tt(nc, dst, u, w, block):
    """dst_hi = u_hi + w_lo ; dst_lo = u_lo - w_hi  per block.
    Arguments are APs."""
    H = block // 2
    vd = dst.rearrange("p (g u) -> p g u", u=block)
    vu = u.rearrange("p (g u) -> p g u", u=block)
    vw = w.rearrange("p (g u) -> p g u", u=block)
    nc.vector.tensor_tensor(vd[:, :, H:], vu[:, :, H:], vw[:, :, :H], ADD)
    nc.vector.tensor_tensor(vd[:, :, :H], vu[:, :, :H], vw[:, :, H:], SUB)


def build_kernel():
    nc = bacc.Bacc()
    x_d = nc.dram_tensor("x", (BSH, N), F32, kind="ExternalInput")
    u_d = nc.dram_tensor("u", (4, 128, W2), BF16, kind="ExternalInput")
    out_d = nc.dram_tensor("out", (BSH, W2), BF16, kind="ExternalOutput")

    with tile.TileContext(nc) as tc:
        with (
            tc.tile_pool(name="const", bufs=1) as const_pool,
            tc.tile_pool(name="wbuild", bufs=2) as wbuild_pool,
            tc.tile_pool(name="wt", bufs=1) as wt_pool,
            tc.tile_pool(name="rot", bufs=2) as rot_pool,
            tc.tile_pool(name="pmm", bufs=2, space=bass.MemorySpace.PSUM) as pmm_pool,
        ):
            # U replicated: Uy k0/k1, U8 k0/k1; split into col-blocks so
            # matmuls can start before the whole matrix lands, and spread
            # the DMAs across queues
            qs = [nc.gpsimd, nc.gpsimd]
            u_sb = []
            for k in range(4):
                blocks = []
                for b in range(NDC):
                    ut = const_pool.tile([128, DC], BF16, tag=f"u{k}b{b}",
                                         name=f"u{k}b{b}")
                    qs[(k + b) % 2].dma_start(ut[:],
                                              u_d[k, :, b * DC:(b + 1) * DC])
                    blocks.append(ut)
                u_sb.append(blocks)

            # x: (1024, 11) -> sbuf (128, 8*11); tile t in cols [t*11,(t+1)*11)
            x_sb = const_pool.tile([128, NTILES * N], F32)
            x_r = x_d.rearrange("(t p) f -> p t f", p=128)
            nc.gpsimd.dma_start(x_sb[:].rearrange("p (t f) -> p t f", f=N), x_r)

            cos_sb = const_pool.tile([128, NTILES * N], F32)
            sin_sb = const_pool.tile([128, NTILES * N], F32)
            hp_t = const_pool.tile([128, 1], F32)
            zr_t = const_pool.tile([128, 1], F32)
            nc.vector.memset(hp_t[:], float(np.pi / 2))
            nc.vector.memset(zr_t[:], 0.0)
            # cos(t) = sin(pi/2 - t): keeps Sin args in (-pi/2, pi/2]
            nc.scalar.activation(cos_sb[:], x_sb[:],
                                 mybir.ActivationFunctionType.Sin,
                                 bias=hp_t[:], scale=-0.5)
            nc.scalar.activation(sin_sb[:], x_sb[:],
                                 mybir.ActivationFunctionType.Sin,
                                 bias=zr_t[:], scale=0.5)

            def csn(t, q):
                col = t * N
                return (cos_sb[:, col + q:col + q + 1],
                        sin_sb[:, col + q:col + q + 1])

            # ---- Phase A: W_lo variants + DMA-crossbar transposes ----
            wts = []
            for t in range(NTILES):
                col = t * N
                wa = wbuild_pool.tile([128, NU], F32, tag="wa")
                wb = wbuild_pool.tile([128, NU], F32, tag="wb")
                nc.vector.tensor_copy(wa[:, 0:1], cos_sb[:, col:col + 1])
                nc.vector.tensor_copy(wa[:, 1:2], sin_sb[:, col:col + 1])
                cur, nxt = wa, wb
                for j in range(1, KLO):
                    half = 1 << j
                    # c-half on ScalarE for the small steps, VectorE rest
                    if half <= 32:
                        nc.scalar.mul(nxt[:, 0:half], cur[:, 0:half],
                                      cos_sb[:, col + j:col + j + 1])
                    else:
                        nc.vector.tensor_scalar_mul(
                            nxt[:, 0:half], cur[:, 0:half],
                            cos_sb[:, col + j:col + j + 1])
                    nc.vector.tensor_scalar_mul(
                        nxt[:, half:2 * half], cur[:, 0:half],
                        sin_sb[:, col + j:col + j + 1])
                    cur, nxt = nxt, cur
                c8, s8 = csn(t, 8)
                wc = wbuild_pool.tile([128, NU], BF16, tag="wc")
                ws = wbuild_pool.tile([128, NU], BF16, tag="ws")
                nc.vector.tensor_scalar_mul(wc[:], cur[:], c8)
                nc.vector.tensor_scalar_mul(ws[:], cur[:], s8)

                wtc = wt_pool.tile([128, NU], BF16, tag=f"wtc{t}")
                wtd = wt_pool.tile([128, NU], BF16, tag=f"wtd{t}")
                tq = [nc.sync, nc.sync, nc.sync, nc.sync]
                for i, (src, dst, k) in enumerate(
                        ((wc, wtc, 0), (wc, wtc, 1), (ws, wtd, 0),
                         (ws, wtd, 1))):
                    tq[i].dma_start_transpose(
                        dst[:, k * 128:(k + 1) * 128],
                        src[:, k * 128:(k + 1) * 128])
                wts.append((wtc, wtd))

            # ---- Phase B: matmuls + rotations per sample-tile ----
            for t in range(NTILES):
                wtc, wtd = wts[t]
                c9, s9 = csn(t, 9)
                c10, s10 = csn(t, 10)

                u9b = rot_pool.tile([128, W2], BF16, tag="u9b")
                w9 = rot_pool.tile([128, W2], BF16, tag="w9")
                T2 = rot_pool.tile([128, W2], BF16, tag="T2")
                u10 = rot_pool.tile([128, W2], BF16, tag="u10")
                w10 = rot_pool.tile([128, W2], BF16, tag="w10")
                T3 = rot_pool.tile([128, W2], BF16, tag="T3")
                for hw in range(2):          # half-row pipeline
                    for dj in range(2):
                        dc = hw * 2 + dj
                        pmm = pmm_pool.tile([128, DC], F32, tag=f"pmm{dc % 2}",
                                            name=f"pmm{dc % 2}")
                        for vi, (wv, ub) in enumerate(
                                ((wtc, 0), (wtc, 1), (wtd, 2), (wtd, 3))):
                            k = ub & 1
                            for h in range(2):
                                nc.tensor.matmul(
                                    pmm[:, h * 512:(h + 1) * 512],
                                    wv[:, k * 128:(k + 1) * 128],
                                    u_sb[ub][dc][:, h * 512:(h + 1) * 512],
                                    start=(vi == 0), stop=(vi == 3))
                        sl = slice(dc * DC, (dc + 1) * DC)
                        nc.scalar.mul(u9b[:, sl], pmm[:], c9)
                        nc.scalar.mul(w9[:, sl], pmm[:], s9)

                    hs = slice(hw * 2048, (hw + 1) * 2048)
                    # r9 (block 32): T2 = u9b +- w9 halves
                    _rot_tt(nc, T2[:, hs], u9b[:, hs], w9[:, hs], 32)
                    # r10 (block 16)
                    nc.vector.tensor_scalar_mul(u10[:, hs], T2[:, hs], c10)
                    nc.vector.tensor_scalar_mul(w10[:, hs], T2[:, hs], s10)
                    _rot_tt(nc, T3[:, hs], u10[:, hs], w10[:, hs], 16)
                    nc.gpsimd.dma_start(out_d[t * 128:(t + 1) * 128, hs],
                                        T3[:, hs])
    nc.finalize()
    return nc


# ----------------------------------------------------------------- driver

_CACHE = {}


def kernel(X, params):
    X = np.ascontiguousarray(np.asarray(X, dtype=np.float32))
    params = np.asarray(params, dtype=np.float32)

    Uy, U8 = build_u_matrices(params)
    u_bf = np.ascontiguousarray(np.stack([
        Uy[:128], Uy[128:], U8[:128], U8[128:],
    ]).astype(ml_dtypes.bfloat16))

    if "nc" not in _CACHE:
        _CACHE["nc"] = build_kernel()
    nc = _CACHE["nc"]

    in_maps = []
    for c in range(NCORES):
        in_maps.append({
            "x": X[c * BSH:(c + 1) * BSH],
            "u": u_bf,
        })
    res = run_bass_kernel_spmd(nc, in_maps, list(range(NCORES)))
    out = np.concatenate([res.results[c]["out"] for c in range(NCORES)],
                         axis=0)
    # device columns are y-ordered; out[x] = dev[y(x)]
    out = out.astype(np.float32).reshape(BATCH, DIM, 2)
    return np.ascontiguousarray(out[:, _y_of_x(), :])


# revision 20
# speedup vs baseline: 1.0527x; 1.0527x over previous
"""Trainium2 Bass kernel for the DataReloadingQNN problem.

Math: layers 0..4 plus the shared RZ/RY/RZ of layer 5 collapse into one
fixed state w (params only).  The data gates are RY(x_q) = c_q I + s_q J_q
with J a signed permutation, all commuting.  Peel qubits 8,9,10: contract
the other eight via a dense matmul T = W_lo @ U with
    W_lo[b, m] = tensor product of 8 [cos,sin] pairs  (m in [0,256)),
    U[m, :]   = (P J^{(m)} w) re/im-interleaved, P = CNOT chain,
then apply per peeled qubit the per-sample rotation
    T <- c_q T + s_q * sign_q ( T[col ^ M_q] ).
Columns are relabeled host-side by a linear GF(2) map chosen so each
peeled rotation is a single y-bit flip with sign = that bit (bits 4,3,2
-> col blocks 64/32/16, contiguous halves).  The q=8 rotation is folded
into the matmul (signed-permuted copy U8, weight variants c8*W and s8*W
-> K=512); q=9 splits as two ScalarE scaled PSUM->SBUF copies (c9*, s9*)
plus two VectorE tensor_tensor half-adds; q=10 as two VectorE
tensor_scalar (4x) plus two tensor_tensor (2x).  W transposes go through
the DMA crossbar, keeping TensorE purely on matmuls.  Output is written
bf16 in y-order; the host upcasts and unpermutes.

Per core: 1024 samples = 8 tiles of 128.  Inputs sharded batch-wise
across 8 cores; U replicated.
"""
import numpy as np
import ml_dtypes

import concourse.bass as bass
import concourse.bacc as bacc
import concourse.tile as tile
from concourse import mybir
from concourse.bass_utils import run_bass_kernel_spmd

N = 11
DIM = 2048
BATCH = 8192
NCORES = 8
BSH = BATCH // NCORES          # 1024 samples per core
NTILES = BSH // 128            # 8 sample-tiles per core
KLO = 8                        # qubits contracted in the matmul
NU = 1 << KLO                  # 256 rows of U
W2 = 2 * DIM                   # 4096 output columns (re/im interleaved)
DC = 1024                      # double-chunk width (2 PSUM banks)
NDC = W2 // DC                 # 4 double-chunks
F32 = mybir.dt.float32
BF16 = mybir.dt.bfloat16

MUL = mybir.AluOpType.mult
ADD = mybir.AluOpType.add
SUB = mybir.AluOpType.subtract

# ---------------------------------------------------------------- host math


def _rz(phi):
    e = np.exp(-0.5j * phi)
    return np.array([[e, 0], [0, np.conj(e)]], dtype=np.complex128)


def _ry(theta):
    t = 0.5 * theta
    c, s = np.cos(t), np.sin(t)
    return np.array([[c, -s], [s, c]], dtype=np.complex128)


def _apply_1q_rows(rows, U, q):
    R = rows.shape[0]
    st = rows.reshape(R, 2 ** q, 2, 2 ** (N - 1 - q))
    st = np.einsum('ab,rxby->rxay', U, st)
    return st.reshape(R, DIM)


def _apply_cnot_rows(rows, c):
    R = rows.shape[0]
    st = rows.reshape(R, 2 ** c, 2, 2, 2 ** (N - 2 - c))
    st = np.stack([st[:, :, 0], st[:, :, 1, ::-1]], axis=2)
    return st.reshape(R, DIM)


def _y_of_x():
    """Column relabeling y = R x: y0=x3, y1=x4, y2=x0^x1, y3=x1^x2,
    y4=x2^x3, y5..10 = x5..x10 (bit i of the state index = 2^i)."""
    x = np.arange(DIM)
    x0, x1 = x & 1, (x >> 1) & 1
    x2, x3 = (x >> 2) & 1, (x >> 3) & 1
    x4 = (x >> 4) & 1
    return ((x & ~np.int64(31)) | (x3 << 0) | (x4 << 1)
            | ((x0 ^ x1) << 2) | ((x1 ^ x2) << 3) | ((x2 ^ x3) << 4))


def _x_of_y():
    y = _y_of_x()
    inv = np.empty(DIM, dtype=np.int64)
    inv[y] = np.arange(DIM)
    return inv


def build_u_matrices(params):
    """(6,11,3) f32 -> (Uy, U8), each (256, 4096) f64 in y-order.
    U8 is the signed bit-4-flip permutation of Uy (folds the q=8 gate)."""
    p = params.astype(np.float64)
    v = np.zeros((1, DIM), dtype=np.complex128)
    v[0, 0] = 1.0
    for l in range(5):
        for q in range(N):
            v = _apply_1q_rows(v, _rz(p[l, q, 0]), q)
            v = _apply_1q_rows(v, _ry(p[l, q, 1]), q)
            v = _apply_1q_rows(v, _rz(p[l, q, 2]), q)
        for c in range(N - 1):
            v = _apply_cnot_rows(v, c)
    for q in range(N):
        B = _rz(p[5, q, 2]) @ _ry(p[5, q, 1]) @ _rz(p[5, q, 0])
        v = _apply_1q_rows(v, B, q)

    # rows over J-subsets of qubits 0..7 (bit b of m <-> qubit b)
    rows = v
    idx = np.arange(DIM)
    for q in range(KLO):
        m = 1 << (N - 1 - q)
        sgn = np.where(idx & m, 1.0, -1.0)
        rows = np.concatenate([rows, sgn * rows[:, idx ^ m]], axis=0)

    # fold CNOT-chain permutation, then relabel columns to y-order
    g = np.arange(DIM)[None, :]
    for c in range(N - 1):
        g = _apply_cnot_rows(g.astype(np.float64), c).astype(np.int64)
    rows = rows[:, g[0]][:, _x_of_y()]

    # fold the q=8 rotation: U8 = sign(y bit 4) * Uy[:, y ^ 16]
    yy = np.arange(DIM)
    sgn8 = np.where((yy >> 4) & 1, 1.0, -1.0)
    rows8 = sgn8[None, :] * rows[:, yy ^ 16]

    def interleave(r):
        U = np.empty((NU, W2), dtype=np.float64)
        U[:, 0::2] = r.real
        U[:, 1::2] = r.imag
        return U

    return interleave(rows), interleave(rows8)


# ------------------------------------------------------------- bass kernel


def _rot_tt(nc, dst, u, w, block):
    """dst_hi = u_hi + w_lo ; dst_lo = u_lo - w_hi  per block.
    Arguments are APs."""
    H = block // 2
    vd = dst.rearrange("p (g u) -> p g u", u=block)
    vu = u.rearrange("p (g u) -> p g u", u=block)
    vw = w.rearrange("p (g u) -> p g u", u=block)
    nc.vector.tensor_tensor(vd[:, :, H:], vu[:, :, H:], vw[:, :, :H], ADD)
    nc.vector.tensor_tensor(vd[:, :, :H], vu[:, :, :H], vw[:, :, H:], SUB)


def build_kernel():
    nc = bacc.Bacc()
    x_d = nc.dram_tensor("x", (BSH, N), F32, kind="ExternalInput")
    u_d = nc.dram_tensor("u", (4, 128, W2), BF16, kind="ExternalInput")
    out_d = nc.dram_tensor("out", (BSH, W2), BF16, kind="ExternalOutput")

    with tile.TileContext(nc) as tc:
        with (
            tc.tile_pool(name="const", bufs=1) as const_pool,
            tc.tile_pool(name="wbuild", bufs=2) as wbuild_pool,
            tc.tile_pool(name="wt", bufs=1) as wt_pool,
            tc.tile_pool(name="rot", bufs=2) as rot_pool,
            tc.tile_pool(name="pmm", bufs=2, space=bass.MemorySpace.PSUM) as pmm_pool,
        ):
            # x first (everything in phase A waits on it), on the fast
            # HWDGE sync queue; U bulk loads on gpsimd
            x_sb = const_pool.tile([128, NTILES * N], F32)
            x_r = x_d.rearrange("(t p) f -> p t f", p=128)
            nc.sync.dma_start(x_sb[:].rearrange("p (t f) -> p t f", f=N), x_r)

            u_sb = []
            for k in range(4):
                ut = const_pool.tile([128, W2], BF16, tag=f"u{k}",
                                     name=f"u{k}")
                nc.gpsimd.dma_start(ut[:], u_d[k])
                u_sb.append(ut)

            cos_sb = const_pool.tile([128, NTILES * N], F32)
            sin_sb = const_pool.tile([128, NTILES * N], F32)
            hp_t = const_pool.tile([128, 1], F32)
            zr_t = const_pool.tile([128, 1], F32)
            nc.vector.memset(hp_t[:], float(np.pi / 2))
            nc.vector.memset(zr_t[:], 0.0)
            # cos(t) = sin(pi/2 - t): keeps Sin args in (-pi/2, pi/2]
            nc.scalar.activation(cos_sb[:], x_sb[:],
                                 mybir.ActivationFunctionType.Sin,
                                 bias=hp_t[:], scale=-0.5)
            nc.scalar.activation(sin_sb[:], x_sb[:],
                                 mybir.ActivationFunctionType.Sin,
                                 bias=zr_t[:], scale=0.5)

            def csn(t, q):
                col = t * N
                return (cos_sb[:, col + q:col + q + 1],
                        sin_sb[:, col + q:col + q + 1])

            # ---- Phase A: W_lo variants + DMA-crossbar transposes ----
            wts = []
            for t in range(NTILES):
                col = t * N
                wa = wbuild_pool.tile([128, NU], F32, tag="wa")
                wb = wbuild_pool.tile([128, NU], F32, tag="wb")
                nc.vector.tensor_copy(wa[:, 0:1], cos_sb[:, col:col + 1])
                nc.vector.tensor_copy(wa[:, 1:2], sin_sb[:, col:col + 1])
                cur, nxt = wa, wb
                for j in range(1, KLO):
                    half = 1 << j
                    # c-half on ScalarE for the small steps, VectorE rest
                    if half <= 32:
                        nc.scalar.mul(nxt[:, 0:half], cur[:, 0:half],
                                      cos_sb[:, col + j:col + j + 1])
                    else:
                        nc.vector.tensor_scalar_mul(
                            nxt[:, 0:half], cur[:, 0:half],
                            cos_sb[:, col + j:col + j + 1])
                    nc.vector.tensor_scalar_mul(
                        nxt[:, half:2 * half], cur[:, 0:half],
                        sin_sb[:, col + j:col + j + 1])
                    cur, nxt = nxt, cur
                c8, s8 = csn(t, 8)
                wc = wbuild_pool.tile([128, NU], BF16, tag="wc")
                ws = wbuild_pool.tile([128, NU], BF16, tag="ws")
                nc.vector.tensor_scalar_mul(wc[:], cur[:], c8)
                nc.vector.tensor_scalar_mul(ws[:], cur[:], s8)

                wv = []
                for i, (src, k) in enumerate(
                        ((wc, 0), (wc, 1), (ws, 0), (ws, 1))):
                    wt = wt_pool.tile([128, 128], BF16, tag=f"wt{t}_{i}",
                                      name=f"wt{t}_{i}")
                    nc.sync.dma_start_transpose(
                        wt[:], src[:, k * 128:(k + 1) * 128])
                    wv.append(wt)
                wts.append(wv)

            # ---- Phase B: matmuls + rotations per sample-tile ----
            for t in range(NTILES):
                wv = wts[t]
                c9, s9 = csn(t, 9)
                c10, s10 = csn(t, 10)

                u9b = rot_pool.tile([128, W2], BF16, tag="u9b")
                w9 = rot_pool.tile([128, W2], BF16, tag="w9")
                T2 = rot_pool.tile([128, W2], BF16, tag="T2")
                u10 = rot_pool.tile([128, W2], BF16, tag="u10")
                w10 = rot_pool.tile([128, W2], BF16, tag="w10")
                T3 = rot_pool.tile([128, W2], BF16, tag="T3")
                for hw in range(2):          # half-row pipeline
                    pmm = pmm_pool.tile([128, 2048], F32, tag="pmm",
                                        name="pmm")
                    for vi, ub in enumerate((0, 1, 2, 3)):
                        for h in range(4):
                            cs = hw * 2048 + h * 512
                            nc.tensor.matmul(
                                pmm[:, h * 512:(h + 1) * 512],
                                wv[vi][:],
                                u_sb[ub][:, cs:cs + 512],
                                start=(vi == 0), stop=(vi == 3))
                    hs = slice(hw * 2048, (hw + 1) * 2048)
                    nc.scalar.mul(u9b[:, hs], pmm[:], c9)
                    nc.scalar.mul(w9[:, hs], pmm[:], s9)

                    # r9 (block 32): T2 = u9b +- w9 halves
                    _rot_tt(nc, T2[:, hs], u9b[:, hs], w9[:, hs], 32)
                    # r10 (block 16)
                    nc.vector.tensor_scalar_mul(u10[:, hs], T2[:, hs], c10)
                    nc.vector.tensor_scalar_mul(w10[:, hs], T2[:, hs], s10)
                    _rot_tt(nc, T3[:, hs], u10[:, hs], w10[:, hs], 16)
                    nc.gpsimd.dma_start(out_d[t * 128:(t + 1) * 128, hs],
                                        T3[:, hs])
    nc.finalize()
    return nc


# ----------------------------------------------------------------- driver

_CACHE = {}


def kernel(X, params):
    X = np.ascontiguousarray(np.asarray(X, dtype=np.float32))
    params = np.asarray(params, dtype=np.float32)

    Uy, U8 = build_u_matrices(params)
    u_bf = np.ascontiguousarray(np.stack([
        Uy[:128], Uy[128:], U8[:128], U8[128:],
    ]).astype(ml_dtypes.bfloat16))

    if "nc" not in _CACHE:
        _CACHE["nc"] = build_kernel()
    nc = _CACHE["nc"]

    in_maps = []
    for c in range(NCORES):
        in_maps.append({
            "x": X[c * BSH:(c + 1) * BSH],
            "u": u_bf,
        })
    res = run_bass_kernel_spmd(nc, in_maps, list(range(NCORES)))
    out = np.concatenate([res.results[c]["out"] for c in range(NCORES)],
                         axis=0)
    # device columns are y-ordered; out[x] = dev[y(x)]
    out = out.astype(np.float32).reshape(BATCH, DIM, 2)
    return np.ascontiguousarray(out[:, _y_of_x(), :])


# revision 21
# speedup vs baseline: 1.0968x; 1.0419x over previous
"""Trainium2 Bass kernel for the DataReloadingQNN problem.

Math: layers 0..4 plus the shared RZ/RY/RZ of layer 5 collapse into one
fixed state w (params only).  The data gates are RY(x_q) = c_q I + s_q J_q
with J a signed permutation, all commuting.  Peel qubits 8,9,10: contract
the other eight via a dense matmul T = W_lo @ U with
    W_lo[b, m] = tensor product of 8 [cos,sin] pairs  (m in [0,256)),
    U[m, :]   = (P J^{(m)} w) re/im-interleaved, P = CNOT chain,
then apply per peeled qubit the per-sample rotation
    T <- c_q T + s_q * sign_q ( T[col ^ M_q] ).
Columns are relabeled host-side by a linear GF(2) map chosen so each
peeled rotation is a single y-bit flip with sign = that bit (bits 4,3,2
-> col blocks 64/32/16, contiguous halves).  The q=8 rotation is folded
into the matmul (signed-permuted copy U8, weight variants c8*W and s8*W
-> K=512); q=9 splits as two ScalarE scaled PSUM->SBUF copies (c9*, s9*)
plus two VectorE tensor_tensor half-adds; q=10 as two VectorE
tensor_scalar (4x) plus two tensor_tensor (2x).  W transposes go through
the DMA crossbar, keeping TensorE purely on matmuls.  Output is written
bf16 in y-order; the host upcasts and unpermutes.

Per core: 1024 samples = 8 tiles of 128.  Inputs sharded batch-wise
across 8 cores; U replicated.
"""
import numpy as np
import ml_dtypes

import concourse.bass as bass
import concourse.bacc as bacc
import concourse.tile as tile
from concourse import mybir
from concourse.bass_utils import run_bass_kernel_spmd

N = 11
DIM = 2048
BATCH = 8192
NCORES = 8
BSH = BATCH // NCORES          # 1024 samples per core
NTILES = BSH // 128            # 8 sample-tiles per core
KLO = 8                        # qubits contracted in the matmul
NU = 1 << KLO                  # 256 rows of U
W2 = 2 * DIM                   # 4096 output columns (re/im interleaved)
DC = 1024                      # double-chunk width (2 PSUM banks)
NDC = W2 // DC                 # 4 double-chunks
F32 = mybir.dt.float32
BF16 = mybir.dt.bfloat16

MUL = mybir.AluOpType.mult
ADD = mybir.AluOpType.add
SUB = mybir.AluOpType.subtract

# ---------------------------------------------------------------- host math


def _rz(phi):
    e = np.exp(-0.5j * phi)
    return np.array([[e, 0], [0, np.conj(e)]], dtype=np.complex128)


def _ry(theta):
    t = 0.5 * theta
    c, s = np.cos(t), np.sin(t)
    return np.array([[c, -s], [s, c]], dtype=np.complex128)


def _apply_1q_rows(rows, U, q):
    R = rows.shape[0]
    st = rows.reshape(R, 2 ** q, 2, 2 ** (N - 1 - q))
    st = np.einsum('ab,rxby->rxay', U, st)
    return st.reshape(R, DIM)


def _apply_cnot_rows(rows, c):
    R = rows.shape[0]
    st = rows.reshape(R, 2 ** c, 2, 2, 2 ** (N - 2 - c))
    st = np.stack([st[:, :, 0], st[:, :, 1, ::-1]], axis=2)
    return st.reshape(R, DIM)


def _y_of_x():
    """Column relabeling y = R x: y0=x3, y1=x4, y2=x0^x1, y3=x1^x2,
    y4=x2^x3, y5..10 = x5..x10 (bit i of the state index = 2^i)."""
    x = np.arange(DIM)
    x0, x1 = x & 1, (x >> 1) & 1
    x2, x3 = (x >> 2) & 1, (x >> 3) & 1
    x4 = (x >> 4) & 1
    return ((x & ~np.int64(31)) | (x3 << 0) | (x4 << 1)
            | ((x0 ^ x1) << 2) | ((x1 ^ x2) << 3) | ((x2 ^ x3) << 4))


def _x_of_y():
    y = _y_of_x()
    inv = np.empty(DIM, dtype=np.int64)
    inv[y] = np.arange(DIM)
    return inv


def build_u_matrices(params):
    """(6,11,3) f32 -> (Uy, U8), each (256, 4096) f64 in y-order.
    U8 is the signed bit-4-flip permutation of Uy (folds the q=8 gate)."""
    p = params.astype(np.float64)
    v = np.zeros((1, DIM), dtype=np.complex128)
    v[0, 0] = 1.0
    for l in range(5):
        for q in range(N):
            v = _apply_1q_rows(v, _rz(p[l, q, 0]), q)
            v = _apply_1q_rows(v, _ry(p[l, q, 1]), q)
            v = _apply_1q_rows(v, _rz(p[l, q, 2]), q)
        for c in range(N - 1):
            v = _apply_cnot_rows(v, c)
    for q in range(N):
        B = _rz(p[5, q, 2]) @ _ry(p[5, q, 1]) @ _rz(p[5, q, 0])
        v = _apply_1q_rows(v, B, q)

    # rows over J-subsets of qubits 0..7 (bit b of m <-> qubit b)
    rows = v
    idx = np.arange(DIM)
    for q in range(KLO):
        m = 1 << (N - 1 - q)
        sgn = np.where(idx & m, 1.0, -1.0)
        rows = np.concatenate([rows, sgn * rows[:, idx ^ m]], axis=0)

    # fold CNOT-chain permutation, then relabel columns to y-order
    g = np.arange(DIM)[None, :]
    for c in range(N - 1):
        g = _apply_cnot_rows(g.astype(np.float64), c).astype(np.int64)
    rows = rows[:, g[0]][:, _x_of_y()]

    # fold the q=8 rotation: U8 = sign(y bit 4) * Uy[:, y ^ 16]
    yy = np.arange(DIM)
    sgn8 = np.where((yy >> 4) & 1, 1.0, -1.0)
    rows8 = sgn8[None, :] * rows[:, yy ^ 16]

    def interleave(r):
        U = np.empty((NU, W2), dtype=np.float64)
        U[:, 0::2] = r.real
        U[:, 1::2] = r.imag
        return U

    return interleave(rows), interleave(rows8)


# ------------------------------------------------------------- bass kernel


def _rot_tt(nc, dst, u, w, block):
    """dst_hi = u_hi + w_lo ; dst_lo = u_lo - w_hi  per block.
    Arguments are APs."""
    H = block // 2
    vd = dst.rearrange("p (g u) -> p g u", u=block)
    vu = u.rearrange("p (g u) -> p g u", u=block)
    vw = w.rearrange("p (g u) -> p g u", u=block)
    nc.vector.tensor_tensor(vd[:, :, H:], vu[:, :, H:], vw[:, :, :H], ADD)
    nc.vector.tensor_tensor(vd[:, :, :H], vu[:, :, :H], vw[:, :, H:], SUB)


def build_kernel():
    nc = bacc.Bacc()
    x_d = nc.dram_tensor("x", (BSH, N), F32, kind="ExternalInput")
    u_d = nc.dram_tensor("u", (4, 128, W2), BF16, kind="ExternalInput")
    out_d = nc.dram_tensor("out", (BSH, W2), BF16, kind="ExternalOutput")

    with tile.TileContext(nc) as tc:
        with (
            tc.tile_pool(name="const", bufs=1) as const_pool,
            tc.tile_pool(name="wbuild", bufs=2) as wbuild_pool,
            tc.tile_pool(name="wt", bufs=1) as wt_pool,
            tc.tile_pool(name="rot", bufs=2) as rot_pool,
            tc.tile_pool(name="pmm", bufs=2, space=bass.MemorySpace.PSUM) as pmm_pool,
        ):
            # x first (everything in phase A waits on it), on the fast
            # HWDGE sync queue; U bulk loads on gpsimd
            x_sb = const_pool.tile([128, NTILES * N], F32)
            x_r = x_d.rearrange("(t p) f -> p t f", p=128)
            nc.sync.dma_start(x_sb[:].rearrange("p (t f) -> p t f", f=N), x_r)

            u_sb = []
            for k in range(4):
                ut = const_pool.tile([128, W2], BF16, tag=f"u{k}",
                                     name=f"u{k}")
                nc.gpsimd.dma_start(ut[:], u_d[k])
                u_sb.append(ut)

            cos_sb = const_pool.tile([128, NTILES * N], F32)
            sin_sb = const_pool.tile([128, NTILES * N], F32)
            hp_t = const_pool.tile([128, 1], F32)
            zr_t = const_pool.tile([128, 1], F32)
            nc.vector.memset(hp_t[:], float(np.pi / 2))
            nc.vector.memset(zr_t[:], 0.0)
            # cos(t) = sin(pi/2 - t): keeps Sin args in (-pi/2, pi/2]
            nc.scalar.activation(cos_sb[:], x_sb[:],
                                 mybir.ActivationFunctionType.Sin,
                                 bias=hp_t[:], scale=-0.5)
            nc.scalar.activation(sin_sb[:], x_sb[:],
                                 mybir.ActivationFunctionType.Sin,
                                 bias=zr_t[:], scale=0.5)

            def csn(t, q):
                col = t * N
                return (cos_sb[:, col + q:col + q + 1],
                        sin_sb[:, col + q:col + q + 1])

            # ---- software-pipelined: W-build of tile t is emitted before
            # phase B of tile t-1 so the DVE queue never stalls the PE ----
            def phase_a(t):
                col = t * N
                wa = wbuild_pool.tile([128, NU], F32, tag="wa")
                wb = wbuild_pool.tile([128, NU], F32, tag="wb")
                nc.vector.tensor_copy(wa[:, 0:1], cos_sb[:, col:col + 1])
                nc.vector.tensor_copy(wa[:, 1:2], sin_sb[:, col:col + 1])
                cur, nxt = wa, wb
                for j in range(1, KLO):
                    half = 1 << j
                    # c-half on ScalarE for the small steps, VectorE rest
                    if half <= 32:
                        nc.scalar.mul(nxt[:, 0:half], cur[:, 0:half],
                                      cos_sb[:, col + j:col + j + 1])
                    else:
                        nc.vector.tensor_scalar_mul(
                            nxt[:, 0:half], cur[:, 0:half],
                            cos_sb[:, col + j:col + j + 1])
                    nc.vector.tensor_scalar_mul(
                        nxt[:, half:2 * half], cur[:, 0:half],
                        sin_sb[:, col + j:col + j + 1])
                    cur, nxt = nxt, cur
                c8, s8 = csn(t, 8)
                wc = wbuild_pool.tile([128, NU], BF16, tag="wc")
                ws = wbuild_pool.tile([128, NU], BF16, tag="ws")
                nc.vector.tensor_scalar_mul(wc[:], cur[:], c8)
                nc.vector.tensor_scalar_mul(ws[:], cur[:], s8)

                wv = []
                for i, (src, k) in enumerate(
                        ((wc, 0), (wc, 1), (ws, 0), (ws, 1))):
                    wt = wt_pool.tile([128, 128], BF16, tag=f"wt{t}_{i}",
                                      name=f"wt{t}_{i}")
                    nc.sync.dma_start_transpose(
                        wt[:], src[:, k * 128:(k + 1) * 128])
                    wv.append(wt)
                return wv

            def phase_b(t, wv):
                c9, s9 = csn(t, 9)
                c10, s10 = csn(t, 10)

                u9b = rot_pool.tile([128, W2], BF16, tag="u9b")
                w9 = rot_pool.tile([128, W2], BF16, tag="w9")
                T2 = rot_pool.tile([128, W2], BF16, tag="T2")
                u10 = rot_pool.tile([128, W2], BF16, tag="u10")
                w10 = rot_pool.tile([128, W2], BF16, tag="w10")
                T3 = rot_pool.tile([128, W2], BF16, tag="T3")
                for hw in range(2):          # half-row pipeline
                    pmm = pmm_pool.tile([128, 2048], F32, tag="pmm",
                                        name="pmm")
                    for vi, ub in enumerate((0, 1, 2, 3)):
                        for h in range(4):
                            cs = hw * 2048 + h * 512
                            nc.tensor.matmul(
                                pmm[:, h * 512:(h + 1) * 512],
                                wv[vi][:],
                                u_sb[ub][:, cs:cs + 512],
                                start=(vi == 0), stop=(vi == 3))
                    hs = slice(hw * 2048, (hw + 1) * 2048)
                    nc.scalar.mul(u9b[:, hs], pmm[:], c9)
                    nc.scalar.mul(w9[:, hs], pmm[:], s9)

                    # r9 (block 32): T2 = u9b +- w9 halves
                    _rot_tt(nc, T2[:, hs], u9b[:, hs], w9[:, hs], 32)
                    # r10 (block 16)
                    nc.vector.tensor_scalar_mul(u10[:, hs], T2[:, hs], c10)
                    nc.vector.tensor_scalar_mul(w10[:, hs], T2[:, hs], s10)
                    _rot_tt(nc, T3[:, hs], u10[:, hs], w10[:, hs], 16)
                    nc.gpsimd.dma_start(out_d[t * 128:(t + 1) * 128, hs],
                                        T3[:, hs])

            wv_prev = phase_a(0)
            for t in range(1, NTILES):
                wv_next = phase_a(t)
                phase_b(t - 1, wv_prev)
                wv_prev = wv_next
            phase_b(NTILES - 1, wv_prev)
    nc.finalize()
    return nc


# ----------------------------------------------------------------- driver

_CACHE = {}


def kernel(X, params):
    X = np.ascontiguousarray(np.asarray(X, dtype=np.float32))
    params = np.asarray(params, dtype=np.float32)

    Uy, U8 = build_u_matrices(params)
    u_bf = np.ascontiguousarray(np.stack([
        Uy[:128], Uy[128:], U8[:128], U8[128:],
    ]).astype(ml_dtypes.bfloat16))

    if "nc" not in _CACHE:
        _CACHE["nc"] = build_kernel()
    nc = _CACHE["nc"]

    in_maps = []
    for c in range(NCORES):
        in_maps.append({
            "x": X[c * BSH:(c + 1) * BSH],
            "u": u_bf,
        })
    res = run_bass_kernel_spmd(nc, in_maps, list(range(NCORES)))
    out = np.concatenate([res.results[c]["out"] for c in range(NCORES)],
                         axis=0)
    # device columns are y-ordered; out[x] = dev[y(x)]
    out = out.astype(np.float32).reshape(BATCH, DIM, 2)
    return np.ascontiguousarray(out[:, _y_of_x(), :])


# revision 22
# speedup vs baseline: 1.1581x; 1.0558x over previous
"""Trainium2 Bass kernel for the DataReloadingQNN problem.

Math: layers 0..4 plus the shared RZ/RY/RZ of layer 5 collapse into one
fixed state w (params only).  The data gates are RY(x_q) = c_q I + s_q J_q
with J a signed permutation, all commuting.  Qubits 0..7 are contracted
by a dense matmul T = W_lo @ U with
    W_lo[b, m] = tensor product of 8 [cos,sin] pairs  (m in [0,256)),
    U[m, :]   = (P J^{(m)} w) re/im-interleaved, P = CNOT chain,
and the peeled qubits 8,9,10 are applied as per-sample rotations
    T <- c_q T + s_q * sign_q ( T[col ^ M_q] ).
Columns are relabeled host-side by a linear GF(2) map chosen so each
peeled rotation is a single y-bit flip with sign = that bit (bits 4,3,2
-> col blocks 64/32/16, contiguous halves).

Device schedule per sample-tile (128 samples):
  - for NPE of the 8 tiles, the q=8 rotation is folded into the matmul
    (signed-permuted copy U8 plus weight variants c8*W, s8*W -> K=512,
    32 matmuls); for the rest it runs on VectorE (K=256, 16 matmuls)
  - ScalarE drains PSUM as two scaled copies (c9*P, s9*P), which is the
    q=9 rotation's prep; VectorE finishes it with two tensor_tensor
    half-adds, then does q=10 (and q=8 on the non-folded tiles) the same
    way (tensor_scalar 4x + tensor_tensor 2x)
  - output is written bf16 in y-order; the host upcasts and unpermutes
W_lo (a 0.2%-of-FLOPs prefix) plus the per-sample cos/sin coefficients
are prepared on the host, pre-scaled and pre-transposed, so the device
does no W build, no transposes and needs no trig tables.

Per core: 1024 samples = 8 tiles of 128.  Inputs sharded batch-wise
across 8 cores; U replicated.
"""
import numpy as np
import ml_dtypes

import concourse.bass as bass
import concourse.bacc as bacc
import concourse.tile as tile
from concourse import mybir
from concourse.bass_utils import run_bass_kernel_spmd

N = 11
DIM = 2048
BATCH = 8192
NCORES = 8
BSH = BATCH // NCORES          # 1024 samples per core
NTILES = BSH // 128            # 8 sample-tiles per core
KLO = 8                        # qubits contracted in the matmul
NU = 1 << KLO                  # 256 rows of U
W2 = 2 * DIM                   # 4096 output columns (re/im interleaved)
NPE = 5                        # tiles with q=8 folded into the matmul
F32 = mybir.dt.float32
BF16 = mybir.dt.bfloat16

ADD = mybir.AluOpType.add
SUB = mybir.AluOpType.subtract

# q=8 on PE for the last NPE tiles; early tiles use only Uy, so matmuls
# can start before U8 lands
TILE_PE = [t >= NTILES - NPE for t in range(NTILES)]

# ---------------------------------------------------------------- host math


def _rz(phi):
    e = np.exp(-0.5j * phi)
    return np.array([[e, 0], [0, np.conj(e)]], dtype=np.complex128)


def _ry(theta):
    t = 0.5 * theta
    c, s = np.cos(t), np.sin(t)
    return np.array([[c, -s], [s, c]], dtype=np.complex128)


def _apply_1q_rows(rows, U, q):
    R = rows.shape[0]
    st = rows.reshape(R, 2 ** q, 2, 2 ** (N - 1 - q))
    st = np.einsum('ab,rxby->rxay', U, st)
    return st.reshape(R, DIM)


def _apply_cnot_rows(rows, c):
    R = rows.shape[0]
    st = rows.reshape(R, 2 ** c, 2, 2, 2 ** (N - 2 - c))
    st = np.stack([st[:, :, 0], st[:, :, 1, ::-1]], axis=2)
    return st.reshape(R, DIM)


def _y_of_x():
    """Column relabeling y = R x: y0=x3, y1=x4, y2=x0^x1, y3=x1^x2,
    y4=x2^x3, y5..10 = x5..x10 (bit i of the state index = 2^i)."""
    x = np.arange(DIM)
    x0, x1 = x & 1, (x >> 1) & 1
    x2, x3 = (x >> 2) & 1, (x >> 3) & 1
    x4 = (x >> 4) & 1
    return ((x & ~np.int64(31)) | (x3 << 0) | (x4 << 1)
            | ((x0 ^ x1) << 2) | ((x1 ^ x2) << 3) | ((x2 ^ x3) << 4))


def _x_of_y():
    y = _y_of_x()
    inv = np.empty(DIM, dtype=np.int64)
    inv[y] = np.arange(DIM)
    return inv


def build_u_matrices(params):
    """(6,11,3) f32 -> (Uy, U8), each (256, 4096) f64 in y-order.
    U8 is the signed bit-4-flip permutation of Uy (folds the q=8 gate)."""
    p = params.astype(np.float64)
    v = np.zeros((1, DIM), dtype=np.complex128)
    v[0, 0] = 1.0
    for l in range(5):
        for q in range(N):
            v = _apply_1q_rows(v, _rz(p[l, q, 0]), q)
            v = _apply_1q_rows(v, _ry(p[l, q, 1]), q)
            v = _apply_1q_rows(v, _rz(p[l, q, 2]), q)
        for c in range(N - 1):
            v = _apply_cnot_rows(v, c)
    for q in range(N):
        B = _rz(p[5, q, 2]) @ _ry(p[5, q, 1]) @ _rz(p[5, q, 0])
        v = _apply_1q_rows(v, B, q)

    # rows over J-subsets of qubits 0..7 (bit b of m <-> qubit b)
    rows = v
    idx = np.arange(DIM)
    for q in range(KLO):
        m = 1 << (N - 1 - q)
        sgn = np.where(idx & m, 1.0, -1.0)
        rows = np.concatenate([rows, sgn * rows[:, idx ^ m]], axis=0)

    # fold CNOT-chain permutation, then relabel columns to y-order
    g = np.arange(DIM)[None, :]
    for c in range(N - 1):
        g = _apply_cnot_rows(g.astype(np.float64), c).astype(np.int64)
    rows = rows[:, g[0]][:, _x_of_y()]

    # fold the q=8 rotation: U8 = sign(y bit 4) * Uy[:, y ^ 16]
    yy = np.arange(DIM)
    sgn8 = np.where((yy >> 4) & 1, 1.0, -1.0)
    rows8 = sgn8[None, :] * rows[:, yy ^ 16]

    def interleave(r):
        U = np.empty((NU, W2), dtype=np.float64)
        U[:, 0::2] = r.real
        U[:, 1::2] = r.imag
        return U

    return interleave(rows), interleave(rows8)


def build_weights(X):
    """Per-sample host prep: cos/sin of x/2, the W_lo tensor product and
    the pre-transposed weight variants [W^T, (c8 W)^T, (s8 W)^T], each
    split into two K-chunks of 128, plus the rotation coefficients."""
    c = np.cos(0.5 * X).astype(np.float64)   # (B, 11)
    s = np.sin(0.5 * X).astype(np.float64)
    B = X.shape[0]
    W = np.ones((B, 1), dtype=np.float64)
    for q in range(KLO):
        W = np.concatenate([W * c[:, q:q + 1], W * s[:, q:q + 1]], axis=1)

    wt = np.empty((3, 2, 128, B), dtype=ml_dtypes.bfloat16)
    for vi, scale in enumerate((np.ones(B), c[:, 8], s[:, 8])):
        Wv = (W * scale[:, None]).astype(ml_dtypes.bfloat16)
        wt[vi, 0] = Wv[:, :128].T
        wt[vi, 1] = Wv[:, 128:].T

    ntile = B // 128
    coef = np.empty((128, ntile * 6), dtype=np.float32)
    for t in range(ntile):
        blk = slice(t * 128, (t + 1) * 128)
        for j, arr in enumerate((c[:, 8], s[:, 8], c[:, 9], s[:, 9],
                                 c[:, 10], s[:, 10])):
            coef[:, t * 6 + j] = arr[blk]
    return wt, coef


# ------------------------------------------------------------- bass kernel


def _rot_tt(nc, dst, u, w, block):
    """dst_hi = u_hi + w_lo ; dst_lo = u_lo - w_hi  per block (APs)."""
    H = block // 2
    vd = dst.rearrange("p (g u) -> p g u", u=block)
    vu = u.rearrange("p (g u) -> p g u", u=block)
    vw = w.rearrange("p (g u) -> p g u", u=block)
    nc.vector.tensor_tensor(vd[:, :, H:], vu[:, :, H:], vw[:, :, :H], ADD)
    nc.vector.tensor_tensor(vd[:, :, :H], vu[:, :, :H], vw[:, :, H:], SUB)


def build_kernel():
    nc = bacc.Bacc()
    wt_d = nc.dram_tensor("wt", (3, 2, 128, BSH), BF16, kind="ExternalInput")
    cf_d = nc.dram_tensor("cf", (128, NTILES * 6), F32, kind="ExternalInput")
    u_d = nc.dram_tensor("u", (4, 128, W2), BF16, kind="ExternalInput")
    out_d = nc.dram_tensor("out", (BSH, W2), BF16, kind="ExternalOutput")

    with tile.TileContext(nc) as tc:
        with (
            tc.tile_pool(name="const", bufs=1) as const_pool,
            tc.tile_pool(name="rot", bufs=2) as rot_pool,
            tc.tile_pool(name="pmm", bufs=2, space=bass.MemorySpace.PSUM) as pmm_pool,
        ):
            # small inputs first on the fast HWDGE sync queue
            cf_sb = const_pool.tile([128, NTILES * 6], F32)
            nc.sync.dma_start(cf_sb[:], cf_d[:])
            wt_of = {}
            for vi in range(3):
                for k in range(2):
                    w = const_pool.tile([128, BSH], BF16, tag=f"wt{vi}{k}",
                                        name=f"wt{vi}{k}")
                    nc.sync.dma_start(w[:], wt_d[vi, k])
                    wt_of[(vi, k)] = w

            # U bulk: Uy first (early tiles need only Uy), U8 on gpsimd
            u_sb = []
            for k in range(4):
                ut = const_pool.tile([128, W2], BF16, tag=f"u{k}",
                                     name=f"u{k}")
                (nc.sync if k < 2 else nc.gpsimd).dma_start(ut[:], u_d[k])
                u_sb.append(ut)

            def cf(t, j):
                return cf_sb[:, t * 6 + j:t * 6 + j + 1]

            for t in range(NTILES):
                ts = slice(t * 128, (t + 1) * 128)
                pe8 = TILE_PE[t]
                if pe8:
                    variants = ((1, 0, 0), (1, 1, 1), (2, 0, 2), (2, 1, 3))
                else:
                    variants = ((0, 0, 0), (0, 1, 1))
                nv = len(variants)

                u9b = rot_pool.tile([128, W2], BF16, tag="u9b")
                w9 = rot_pool.tile([128, W2], BF16, tag="w9")
                T2 = rot_pool.tile([128, W2], BF16, tag="T2")
                ua = rot_pool.tile([128, W2], BF16, tag="ua")
                wa = rot_pool.tile([128, W2], BF16, tag="wa")
                T3 = rot_pool.tile([128, W2], BF16, tag="T3")
                if not pe8:
                    T4 = rot_pool.tile([128, W2], BF16, tag="T4")
                    ub = rot_pool.tile([128, W2], BF16, tag="ub")
                    wb = rot_pool.tile([128, W2], BF16, tag="wb")

                for hw in range(2):          # half-row pipeline
                    pmm = pmm_pool.tile([128, 2048], F32, tag="pmm",
                                        name="pmm")
                    for vi, (wvar, k, ui) in enumerate(variants):
                        wop = wt_of[(wvar, k)]
                        for h in range(4):
                            cs = hw * 2048 + h * 512
                            nc.tensor.matmul(
                                pmm[:, h * 512:(h + 1) * 512],
                                wop[:, ts],
                                u_sb[ui][:, cs:cs + 512],
                                start=(vi == 0), stop=(vi == nv - 1))
                    hs = slice(hw * 2048, (hw + 1) * 2048)
                    # q=9 prep fused into the PSUM drain
                    nc.scalar.mul(u9b[:, hs], pmm[:], cf(t, 2))
                    nc.scalar.mul(w9[:, hs], pmm[:], cf(t, 3))

                    # q=9 (block 32)
                    _rot_tt(nc, T2[:, hs], u9b[:, hs], w9[:, hs], 32)
                    if not pe8:
                        # q=8 on VectorE (block 64)
                        nc.vector.tensor_scalar_mul(ub[:, hs], T2[:, hs],
                                                    cf(t, 0))
                        nc.vector.tensor_scalar_mul(wb[:, hs], T2[:, hs],
                                                    cf(t, 1))
                        _rot_tt(nc, T4[:, hs], ub[:, hs], wb[:, hs], 64)
                        src = T4
                    else:
                        src = T2
                    # q=10 (block 16)
                    nc.vector.tensor_scalar_mul(ua[:, hs], src[:, hs],
                                                cf(t, 4))
                    nc.vector.tensor_scalar_mul(wa[:, hs], src[:, hs],
                                                cf(t, 5))
                    _rot_tt(nc, T3[:, hs], ua[:, hs], wa[:, hs], 16)
                    nc.gpsimd.dma_start(out_d[ts, hs], T3[:, hs])
    nc.finalize()
    return nc


# ----------------------------------------------------------------- driver

_CACHE = {}


def kernel(X, params):
    X = np.ascontiguousarray(np.asarray(X, dtype=np.float32))
    params = np.asarray(params, dtype=np.float32)

    Uy, U8 = build_u_matrices(params)
    u_bf = np.ascontiguousarray(np.stack([
        Uy[:128], Uy[128:], U8[:128], U8[128:],
    ]).astype(ml_dtypes.bfloat16))
    wt, coef = build_weights(X)

    if "nc" not in _CACHE:
        _CACHE["nc"] = build_kernel()
    nc = _CACHE["nc"]

    ncols = BATCH // 128
    coef3 = coef.reshape(128, ncols, 6)
    in_maps = []
    for c in range(NCORES):
        bs = slice(c * BSH, (c + 1) * BSH)
        in_maps.append({
            "wt": np.ascontiguousarray(wt[:, :, :, bs]),
            "cf": np.ascontiguousarray(
                coef3[:, c * NTILES:(c + 1) * NTILES].reshape(
                    128, NTILES * 6)),
            "u": u_bf,
        })
    res = run_bass_kernel_spmd(nc, in_maps, list(range(NCORES)))
    out = np.concatenate([res.results[c]["out"] for c in range(NCORES)],
                         axis=0)
    # device columns are y-ordered; out[x] = dev[y(x)]
    out = out.astype(np.float32).reshape(BATCH, DIM, 2)
    return np.ascontiguousarray(out[:, _y_of_x(), :])
